# revision 1
# baseline (speedup 1.0000x reference)
"""3-layer GCN + global mean pool, distributed over 8 Trainium2 NeuronCores.

Strategy (see spec sharding hint):
- Nodes are partitioned into 8 contiguous shards (12500 real nodes each,
  padded to 12544 = 98 tiles of 128).
- Per layer: every core computes u = dinv * (h @ W) for its shard on the
  TensorEngine, the u-table is AllGathered to every core's HBM, and each
  core gathers u[src] rows (dma_gather, int16 indices -> 4 source blocks)
  for the edges whose dst lives in its shard.  The segment-sum over
  incoming edges is done as a sequence of one-hot ("staircase") matmuls
  accumulating in PSUM: stair[e, v] = (dstslot[e] == v), generated on the
  fly on the DVE by comparing an iota row against per-edge dst slots.
  Padding edges carry dstslot = -1 and thus contribute zero.
- Mean pool: per node tile, a [128, 1024] one-hot over global graph ids
  feeds 8 accumulating matmuls into a [128, 1024] PSUM tile; partial sums
  are AllReduced across cores, divided by counts, and pushed through the
  final linear layer (replicated on every core).

The edge structure (chunk counts per tile/block) is made uniform across
cores so a single SPMD program serves all 8 cores; per-core data
(indices, dst slots, degrees, batch slots) is shipped as input tensors.
"""

import math

import numpy as np

import concourse.bacc as bacc
import concourse.bass as bass
import concourse.mybir as mybir
import concourse.tile as tile
from concourse.bass_utils import run_bass_kernel_spmd

P = 128
NCORES = 8
F32 = mybir.dt.float32
I16 = mybir.dt.int16


def _ceil_div(a, b):
    return (a + b - 1) // b


def _preprocess(x, edge_index, batch, n_graphs, group_tiles=4, maxidx=32767):
    """Build per-core input tensors + uniform static structure (meta)."""
    N = x.shape[0]
    DIN = x.shape[1]
    SR = N // NCORES              # real nodes per shard
    assert SR * NCORES == N
    TPC = _ceil_div(SR, P)        # tiles per core
    S = TPC * P                   # padded shard rows
    BLKS = maxidx // S            # shards per index block (int16 range)
    NBLK = _ceil_div(NCORES, BLKS)
    BLKR = BLKS * S               # rows per index block
    GT = group_tiles
    NG = _ceil_div(TPC, GT)
    GP = _ceil_div(n_graphs, P) * P   # padded graph count (1024)
    NGT = GP // P

    # self-loops are NOT materialized as edges: the tile's own u is added
    # algebraically on-device (saves one gather index per node per layer).
    src = np.asarray(edge_index[0], dtype=np.int64)
    dst = np.asarray(edge_index[1], dtype=np.int64)
    deg = (np.bincount(dst, minlength=N) + 1).astype(np.float32)  # + self

    src_row = (src // SR) * S + (src % SR)     # row in the gathered u table
    dst_core = dst // SR
    dst_slot = dst % SR

    batch = np.asarray(batch, dtype=np.int64)

    # ---- per (core, tile, block) edge counts, then uniform chunk counts
    tile_of = dst_slot // P
    blk_of = src_row // BLKR
    counts = np.zeros((NCORES, TPC, NBLK), dtype=np.int64)
    np.add.at(counts, (dst_core, tile_of, blk_of), 1)
    ch = _ceil_div(counts.max(axis=0), 1)  # per-core max below
    ch = np.ceil(counts.max(axis=0) / P).astype(np.int64)      # [TPC, NBLK]
    ch = np.maximum(ch, (counts.max(axis=0) > 0))              # safety
    ch[:, 0] = np.maximum(ch[:, 0], 1)   # every tile gets >=1 chunk
    TOTCH = int(ch.sum())

    # ---- group/chunk layout (uniform across cores)
    # msg-buffer chunks are laid out block-major (to match the per-block
    # gather calls); dslot columns are laid out TILE-major so each tile's
    # staircase op reads a contiguous dslot slice.  tmbase[t] is the
    # tile-major dslot column base for tile t.
    groups = []
    chunk_cursor = 0   # tile-major dslot column cursor
    idx_cursor16 = 0
    tmbase = {}
    for t in range(TPC):
        tmbase[t] = chunk_cursor
        chunk_cursor += int(ch[t, :].sum())
    assert chunk_cursor == TOTCH
    for g in range(NG):
        tiles = list(range(g * GT, min((g + 1) * GT, TPC)))
        blocks = []
        tile_chunks = {t: [] for t in tiles}   # local msg-chunk ids, in
        local = 0                              # tile-major emission order
        for b in range(NBLK):
            nb = int(sum(ch[t, b] for t in tiles))
            blocks.append((idx_cursor16, nb * P, local))
            for t in tiles:
                for _ in range(int(ch[t, b])):
                    tile_chunks[t].append(local)
                    local += 1
            idx_cursor16 += nb * P // 16
        groups.append({
            "tiles": [(t, tile_chunks[t], tmbase[t]) for t in tiles],
            "blocks": blocks,
            "n_chunks": local,
        })
    TOT16 = idx_cursor16
    CHMAX = max(g["n_chunks"] for g in groups)

    # ---- per-core tensors
    in_maps = []
    # order edges once per core by (tile, block); stable order inside
    for c in range(NCORES):
        m = dst_core == c
        e_row = src_row[m]
        e_tile = tile_of[m]
        e_blk = blk_of[m]
        e_slot = (dst_slot[m] % P).astype(np.float32)
        order = np.lexsort((e_blk, e_tile))
        e_row, e_tile, e_blk, e_slot = (
            e_row[order], e_tile[order], e_blk[order], e_slot[order])
        # bucket boundaries
        key = e_tile * NBLK + e_blk
        bnd = np.searchsorted(key, np.arange(TPC * NBLK + 1))
        idx_vals = np.zeros(TOT16 * 16, dtype=np.int16)
        dslot_vals = np.full(TOTCH * P, -1.0, dtype=np.float32)
        pos = 0   # position in idx_vals stream (gather order: block-major)
        dpos = {t: 0 for t in range(TPC)}   # per-tile dslot chunks written
        for gi, g in enumerate(groups):
            for b_i, (off16, n_idx, local0) in enumerate(g["blocks"]):
                assert pos == off16 * 16
                for (t, _tch, tmb) in g["tiles"]:
                    k = t * NBLK + b_i
                    lo, hi = bnd[k], bnd[k + 1]
                    n_pad = int(ch[t, b_i]) * P
                    seg_idx = np.zeros(n_pad, dtype=np.int16)
                    seg_idx[: hi - lo] = (e_row[lo:hi] % BLKR).astype(np.int16)
                    seg_slot = np.full(n_pad, -1.0, dtype=np.float32)
                    seg_slot[: hi - lo] = e_slot[lo:hi]
                    idx_vals[pos: pos + n_pad] = seg_idx
                    # dslot goes to the tile-major column range
                    d0 = (tmb + dpos[t]) * P
                    dslot_vals[d0: d0 + n_pad] = seg_slot
                    dpos[t] += int(ch[t, b_i])
                    pos += n_pad
        assert pos == TOT16 * 16
        # wrap indices: idx i -> [i % 16, i // 16], replicated to 128 rows
        idx_w = idx_vals.reshape(-1, 16).T.copy()          # [16, TOT16]
        idx_w = np.tile(idx_w, (8, 1))                     # [128, TOT16]
        # dslot: [128, TOTCH] : chunk j partition p = edge j*128+p
        dslot_w = dslot_vals.reshape(TOTCH, P).T.copy()

        nodes = np.arange(c * SR, (c + 1) * SR)
        deg_flat = np.ones(S, dtype=np.float32)
        deg_flat[:SR] = deg[nodes]
        deg_w = deg_flat.reshape(TPC, P).T.copy()
        pool_flat = np.full(S, -1.0, dtype=np.float32)
        pool_flat[:SR] = batch[nodes].astype(np.float32)
        pool_w = pool_flat.reshape(TPC, P).T.copy()

        xT = np.zeros((DIN, S), dtype=np.float32)
        xT[:, :SR] = np.asarray(x[nodes], dtype=np.float32).T

        cnt = np.bincount(batch, minlength=n_graphs).astype(np.float32)
        cnt_flat = np.ones(GP, dtype=np.float32)
        cnt_flat[:n_graphs] = cnt
        cnt_w = cnt_flat.reshape(NGT, P).T.copy()

        iota = np.broadcast_to(
            np.arange(GP, dtype=np.float32)[None, :], (P, GP)).copy()

        in_maps.append({
            "xT": xT, "idx": idx_w, "dslot": dslot_w, "deg": deg_w,
            "pslot": pool_w, "cnt": cnt_w, "iota": iota,
        })

    meta = dict(N=N, DIN=DIN, SR=SR, S=S, TPC=TPC, NBLK=NBLK, BLKR=BLKR,
                GT=GT, NG=NG, GP=GP, NGT=NGT, TOTCH=TOTCH, TOT16=TOT16,
                CHMAX=CHMAX, groups=groups, n_graphs=n_graphs)
    return in_maps, meta


def _build(meta, weights, msg_bufs=2, stage=4, use_bf16=False):
    """Build the SPMD Bass program. weights: dict of numpy arrays (same on
    every core) -> shipped as inputs."""
    SR, S, TPC = meta["SR"], meta["S"], meta["TPC"]
    NBLK, BLKR = meta["NBLK"], meta["BLKR"]
    GP, NGT = meta["GP"], meta["NGT"]
    TOTCH, TOT16, CHMAX = meta["TOTCH"], meta["TOT16"], meta["CHMAX"]
    DIN = meta["DIN"]
    H = weights["W1"].shape[1]
    OUT = weights["Wl"].shape[1]
    n_graphs = meta["n_graphs"]
    has_b = [bool(np.any(weights[k])) for k in ("b1", "b2", "b3")]
    has_bl = bool(np.any(weights["bl"]))

    DT = mybir.dt.bfloat16 if use_bf16 else F32
    nc = bacc.Bacc("TRN2", target_bir_lowering=False, debug=False,
                   num_devices=NCORES)

    # ---- I/O tensors
    t_xT = nc.dram_tensor("xT", [DIN, S], F32, kind="ExternalInput")
    t_idx = nc.dram_tensor("idx", [P, TOT16], I16, kind="ExternalInput")
    t_dslot = nc.dram_tensor("dslot", [P, TOTCH], F32, kind="ExternalInput")
    t_deg = nc.dram_tensor("deg", [P, TPC], F32, kind="ExternalInput")
    t_pslot = nc.dram_tensor("pslot", [P, TPC], F32, kind="ExternalInput")
    t_cnt = nc.dram_tensor("cnt", [P, NGT], F32, kind="ExternalInput")
    t_iota = nc.dram_tensor("iota", [P, GP], F32, kind="ExternalInput")
    t_W = {}
    for wn, shp in (("W1", [DIN, H]), ("W2", [H, H]), ("W3", [H, H]),
                    ("Wl", [H, OUT])):
        t_W[wn] = nc.dram_tensor(wn, shp, F32, kind="ExternalInput")
    t_b = {}
    for bn in ("b1", "b2", "b3"):
        t_b[bn] = nc.dram_tensor(bn, [P, H], F32, kind="ExternalInput")
    t_bl = nc.dram_tensor("bl", [P, OUT], F32, kind="ExternalInput")
    t_out = nc.dram_tensor("out", [n_graphs, OUT], F32, kind="ExternalOutput")

    AOP = mybir.AluOpType
    ACT = mybir.ActivationFunctionType

    with tile.TileContext(nc, num_cores=NCORES) as tc:
        with tc.tile_pool(name="const", bufs=1) as cp, \
             tc.tile_pool(name="dram", bufs=1, space="DRAM") as dp:
            # ---- constants to SBUF
            iota_sb = cp.tile([P, GP], F32)
            nc.sync.dma_start(iota_sb[:], t_iota[:])
            idx_sb = cp.tile([P, TOT16], I16)
            nc.sync.dma_start(idx_sb[:], t_idx[:])
            dslot_sb = cp.tile([P, TOTCH], F32)
            nc.sync.dma_start(dslot_sb[:], t_dslot[:])
            deg_sb = cp.tile([P, TPC], F32)
            nc.sync.dma_start(deg_sb[:], t_deg[:])
            pslot_sb = cp.tile([P, TPC], F32)
            nc.sync.dma_start(pslot_sb[:], t_pslot[:])
            cnt_sb = cp.tile([P, NGT], F32)
            nc.sync.dma_start(cnt_sb[:], t_cnt[:])
            W_sb = {}
            for wn, t_w in t_W.items():
                W_sb[wn] = cp.tile(list(t_w.shape), F32, name=f"W_{wn}_sb")
                nc.sync.dma_start(W_sb[wn][:], t_w[:])
            b_sb = {}
            for i, bn in enumerate(("b1", "b2", "b3")):
                if has_b[i]:
                    b_sb[bn] = cp.tile([P, H], F32, name=f"b_{bn}_sb")
                    nc.sync.dma_start(b_sb[bn][:], t_b[bn][:])
            if has_bl:
                bl_sb = cp.tile([P, OUT], F32)
                nc.sync.dma_start(bl_sb[:], t_bl[:])
            ident_sb = cp.tile([P, P], F32)
            from concourse.masks import make_identity
            make_identity(nc, ident_sb[:])

            # dinv = 1/sqrt(deg)  (deg >= 1 always: self loops)
            dinv_sb = cp.tile([P, TPC], F32)
            nc.scalar.sqrt(dinv_sb[:], deg_sb[:])
            nc.vector.reciprocal(dinv_sb[:], dinv_sb[:])
            # cntinv = 1/max(cnt, 1)
            cntinv_sb = cp.tile([P, NGT], F32)
            nc.vector.tensor_scalar_max(cntinv_sb[:], cnt_sb[:], 1.0)
            nc.vector.reciprocal(cntinv_sb[:], cntinv_sb[:])

            # ---- DRAM scratch
            u_shard = dp.tile([S, H], DT)
            u_table = dp.tile([S * NCORES, H], DT)
            pool_dram = dp.tile([GP, H], F32)
            pool_ar = dp.tile([GP, H], F32)

            # ---- layer-1 u: u1 = dinv * (x @ W1), from xT shipped by host
            with tc.tile_pool(name="xTp", bufs=1) as xp, \
                 tc.tile_pool(name="u1w", bufs=4) as u1p, \
                 tc.tile_pool(name="u1ps", bufs=4, space="PSUM") as u1ps:
                xT_sb = xp.tile([DIN, S], F32)
                nc.sync.dma_start(xT_sb[:], t_xT[:])
                for t in range(TPC):
                    ps = u1ps.tile([P, H], F32, tag="ps")
                    nc.tensor.matmul(ps[:], lhsT=xT_sb[:, t * P:(t + 1) * P],
                                     rhs=W_sb["W1"][:], start=True, stop=True)
                    u_sb = u1p.tile([P, H], DT, tag="u")
                    nc.scalar.activation(u_sb[:], ps[:], ACT.Copy,
                                         scale=dinv_sb[:, t:t + 1])
                    nc.sync.dma_start(u_shard[t * P:(t + 1) * P, :], u_sb[:])

            # ---- main layer loop
            with tc.tile_pool(name="msg", bufs=msg_bufs) as mp, \
                 tc.tile_pool(name="stair", bufs=2) as sp, \
                 tc.tile_pool(name="work", bufs=3) as wp, \
                 tc.tile_pool(name="hps", bufs=2, space="PSUM") as hps, \
                 tc.tile_pool(name="tps", bufs=2, space="PSUM") as tps, \
                 tc.tile_pool(name="ups", bufs=2, space="PSUM") as ups, \
                 tc.tile_pool(name="pps", bufs=1, space="PSUM") as pps:
                pool_ps = pps.tile([P, GP], F32)
                n_layers = 3 if stage >= 3 else (1 if stage >= 1.5 else 0)
                for li in range(n_layers):
                    # AllGather this layer's u
                    nc.gpsimd.collective_compute(
                        "AllGather", AOP.bypass,
                        replica_groups=[list(range(NCORES))],
                        ins=[u_shard.opt()], outs=[u_table.opt()],
                    )
                    W_next = ("W2", "W3", None)[li]
                    for g in meta["groups"]:
                        nch = g["n_chunks"]
                        msg = mp.tile([P, CHMAX, H], DT, tag="msg")
                        for b_i, (off16, n_idx, local0) in enumerate(g["blocks"]):
                            if n_idx == 0 or stage < 2:
                                continue
                            nbch = n_idx // P
                            blk_hi = min((b_i + 1) * BLKR, S * NCORES)
                            nc.gpsimd.dma_gather(
                                out_ap=msg[:, local0:local0 + nbch, :],
                                in_ap=u_table[b_i * BLKR:blk_hi, :],
                                idxs_ap=idx_sb[:, off16:off16 + n_idx // 16],
                                num_idxs=n_idx,
                                num_idxs_reg=n_idx,
                                elem_size=H,
                                single_packet=False,
                            )
                        for (t, tch, tmb) in g["tiles"]:
                            if not tch or stage < 3:
                                continue
                            ntc = len(tch)
                            # one-hot staircases for all chunks of this tile
                            stair = sp.tile([P, ntc, P], DT, tag="st")
                            nc.vector.tensor_tensor(
                                out=stair[:],
                                in0=iota_sb[:, 0:P].unsqueeze(1)
                                    .to_broadcast([P, ntc, P]),
                                in1=dslot_sb[:, tmb:tmb + ntc].unsqueeze(2)
                                    .to_broadcast([P, ntc, P]),
                                op=AOP.is_equal,
                            )
                            ps_h = hps.tile([P, H], F32, tag="h")
                            for k, lc in enumerate(tch):
                                nc.tensor.matmul(
                                    ps_h[:], lhsT=stair[:, k, :],
                                    rhs=msg[:, lc, :],
                                    start=(k == 0), stop=(k == ntc - 1))
                            # self-loop: add this tile's own u (local read)
                            u_loc = wp.tile([P, H], DT, tag="uloc")
                            nc.sync.dma_start(u_loc[:],
                                              u_shard[t * P:(t + 1) * P, :])
                            tmp = wp.tile([P, H], F32, tag="tmp")
                            nc.vector.tensor_tensor(
                                out=tmp[:], in0=ps_h[:], in1=u_loc[:],
                                op=AOP.add)
                            h_sb = wp.tile([P, H], F32, tag="h")
                            if has_b[li]:
                                nc.vector.tensor_scalar_mul(
                                    tmp[:], tmp[:], dinv_sb[:, t:t + 1])
                                nc.vector.tensor_tensor(
                                    out=tmp[:], in0=tmp[:],
                                    in1=b_sb[("b1", "b2", "b3")[li]][:],
                                    op=AOP.add)
                                nc.scalar.activation(h_sb[:], tmp[:], ACT.Relu)
                            else:
                                nc.scalar.activation(
                                    h_sb[:], tmp[:], ACT.Relu,
                                    scale=dinv_sb[:, t:t + 1])
                            if W_next is not None:
                                ps_t = tps.tile([P, P], F32, tag="t")
                                nc.tensor.transpose(ps_t[:], h_sb[:],
                                                    ident_sb[:])
                                hT_sb = wp.tile([P, P], F32, tag="ht")
                                nc.vector.tensor_copy(hT_sb[:], ps_t[:])
                                ps_u = ups.tile([P, H], F32, tag="u")
                                nc.tensor.matmul(ps_u[:], lhsT=hT_sb[:],
                                                 rhs=W_sb[W_next][:],
                                                 start=True, stop=True)
                                u_sb = wp.tile([P, H], DT, tag="u")
                                nc.scalar.activation(
                                    u_sb[:], ps_u[:], ACT.Copy,
                                    scale=dinv_sb[:, t:t + 1])
                                nc.sync.dma_start(
                                    u_shard[t * P:(t + 1) * P, :], u_sb[:])
                            else:
                                # pool: one-hot over global graph slots
                                stp = sp.tile([P, GP], F32, tag="stp")
                                nc.vector.tensor_tensor(
                                    out=stp[:], in0=iota_sb[:],
                                    in1=pslot_sb[:, t:t + 1]
                                        .to_broadcast([P, GP]),
                                    op=AOP.is_equal)
                                # start=True clears has_written for the WHOLE
                                # 2KB psum bank -> only the first matmul per
                                # bank may set it (4 halves of 128 f32 / bank).
                                for hh in range(NGT):
                                    nc.tensor.matmul(
                                        pool_ps[:, hh * P:(hh + 1) * P],
                                        lhsT=stp[:, hh * P:(hh + 1) * P],
                                        rhs=h_sb[:],
                                        start=(t == 0 and hh % 4 == 0),
                                        stop=(t == TPC - 1),
                                        skip_group_check=True)

                if stage < 4:
                    z_sb = wp.tile([P, OUT], F32, tag="o")
                    nc.vector.memset(z_sb[:], 0.0)
                    nc.sync.dma_start(t_out[0:min(P, n_graphs), :],
                                      z_sb[:min(P, n_graphs), :])
                else:
                    # ---- pool wrap-up: PSUM -> SBUF -> DRAM -> AllReduce
                    poolacc = wp.tile([P, GP], F32, tag="pa")
                    nc.vector.tensor_copy(poolacc[:], pool_ps[:])
                    nc.sync.dma_start(
                        pool_dram[:].rearrange("(h p) f -> p h f", p=P),
                        poolacc[:].rearrange("p (h f) -> p h f", h=NGT))
                    nc.gpsimd.collective_compute(
                        "AllReduce", AOP.add,
                        replica_groups=[list(range(NCORES))],
                        ins=[pool_dram.opt()], outs=[pool_ar.opt()],
                    )
                    # ---- final linear on pooled means (replicated)
                    for gt in range(NGT):
                        pt = wp.tile([P, H], F32, tag="pt")
                        nc.sync.dma_start(pt[:],
                                          pool_ar[gt * P:(gt + 1) * P, :])
                        nc.vector.tensor_scalar_mul(pt[:], pt[:],
                                                    cntinv_sb[:, gt:gt + 1])
                        ps_t = tps.tile([P, P], F32, tag="t")
                        nc.tensor.transpose(ps_t[:], pt[:], ident_sb[:])
                        ptT = wp.tile([P, P], F32, tag="ptT")
                        nc.vector.tensor_copy(ptT[:], ps_t[:])
                        ps_o = ups.tile([P, OUT], F32, tag="u")
                        nc.tensor.matmul(ps_o[:], lhsT=ptT[:],
                                         rhs=W_sb["Wl"][:],
                                         start=True, stop=True)
                        o_sb = wp.tile([P, OUT], F32, tag="o")
                        if has_bl:
                            nc.vector.tensor_tensor(out=o_sb[:], in0=ps_o[:],
                                                    in1=bl_sb[:], op=AOP.add)
                        else:
                            nc.vector.tensor_copy(o_sb[:], ps_o[:])
                        rows = min(P, n_graphs - gt * P)
                        nc.sync.dma_start(t_out[gt * P:gt * P + rows, :],
                                          o_sb[:rows, :])

    nc.finalize()
    return nc


def kernel(x, edge_index, batch, W1, b1, W2, b2, W3, b3, Wl, bl,
           group_tiles=4, trace=False, n_graphs=1000, stage=4, use_bf16=True):
    weights = dict(W1=np.asarray(W1, np.float32), b1=np.asarray(b1, np.float32),
                   W2=np.asarray(W2, np.float32), b2=np.asarray(b2, np.float32),
                   W3=np.asarray(W3, np.float32), b3=np.asarray(b3, np.float32),
                   Wl=np.asarray(Wl, np.float32), bl=np.asarray(bl, np.float32))
    in_maps, meta = _preprocess(np.asarray(x, np.float32),
                                np.asarray(edge_index), np.asarray(batch),
                                n_graphs, group_tiles=group_tiles)
    nc = _build(meta, weights, stage=stage, use_bf16=use_bf16)
    # broadcast weight tensors (same on every core)
    H = weights["W1"].shape[1]
    OUT = weights["Wl"].shape[1]
    for m in in_maps:
        for wn in ("W1", "W2", "W3", "Wl"):
            m[wn] = weights[wn]
        for bn in ("b1", "b2", "b3"):
            m[bn] = np.broadcast_to(weights[bn][None, :], (P, H)).copy()
        m["bl"] = np.broadcast_to(weights["bl"][None, :], (P, OUT)).copy()
    res = run_bass_kernel_spmd(nc, in_maps, core_ids=list(range(NCORES)),
                               trace=trace)
    kernel.last_result = res
    return res.results[0]["out"][:n_graphs].astype(np.float32)



# revision 4
# speedup vs baseline: 2.2263x; 2.2263x over previous
"""3-layer GCN + global mean pool, distributed over 8 Trainium2 NeuronCores.

Strategy (see spec sharding hint):
- Nodes are partitioned into 8 contiguous shards (12500 real nodes each,
  padded to 12544 = 98 tiles of 128).
- Per layer: every core computes u = dinv * (h @ W) for its shard on the
  TensorEngine, the u-table is AllGathered to every core's HBM, and each
  core gathers u[src] rows (dma_gather, int16 indices -> 4 source blocks)
  for the edges whose dst lives in its shard.  The segment-sum over
  incoming edges is done as a sequence of one-hot ("staircase") matmuls
  accumulating in PSUM: stair[e, v] = (dstslot[e] == v), generated on the
  fly on the DVE by comparing an iota row against per-edge dst slots.
  Padding edges carry dstslot = -1 and thus contribute zero.
- Mean pool: per node tile, a [128, 1024] one-hot over global graph ids
  feeds 8 accumulating matmuls into a [128, 1024] PSUM tile; partial sums
  are AllReduced across cores, divided by counts, and pushed through the
  final linear layer (replicated on every core).

The edge structure (chunk counts per tile/block) is made uniform across
cores so a single SPMD program serves all 8 cores; per-core data
(indices, dst slots, degrees, batch slots) is shipped as input tensors.
"""

import math

import numpy as np

import concourse.bacc as bacc
import concourse.bass as bass
import concourse.mybir as mybir
import concourse.tile as tile
from concourse.bass_utils import run_bass_kernel_spmd

P = 128
NCORES = 8
F32 = mybir.dt.float32
I16 = mybir.dt.int16


def _ceil_div(a, b):
    return (a + b - 1) // b


def _preprocess(x, edge_index, batch, n_graphs, group_tiles=4, maxidx=32767):
    """Build per-core input tensors + uniform static structure (meta)."""
    N = x.shape[0]
    DIN = x.shape[1]
    SR = N // NCORES              # real nodes per shard
    assert SR * NCORES == N
    TPC = _ceil_div(SR, P)        # tiles per core
    S = TPC * P                   # padded shard rows
    BLKS = maxidx // S            # shards per index block (int16 range)
    NBLK = _ceil_div(NCORES, BLKS)
    BLKR = BLKS * S               # rows per index block
    GT = group_tiles
    NG = _ceil_div(TPC, GT)
    GP = _ceil_div(n_graphs, P) * P   # padded graph count (1024)
    NGT = GP // P

    # self-loops are NOT materialized as edges: the tile's own u is added
    # algebraically on-device (saves one gather index per node per layer).
    src = np.asarray(edge_index[0], dtype=np.int64)
    dst = np.asarray(edge_index[1], dtype=np.int64)
    deg = (np.bincount(dst, minlength=N) + 1).astype(np.float32)  # + self

    src_row = (src // SR) * S + (src % SR)     # row in the gathered u table
    dst_core = dst // SR
    dst_slot = dst % SR

    batch = np.asarray(batch, dtype=np.int64)

    # ---- per (core, tile, block) edge counts, then uniform chunk counts
    tile_of = dst_slot // P
    blk_of = src_row // BLKR
    counts = np.zeros((NCORES, TPC, NBLK), dtype=np.int64)
    np.add.at(counts, (dst_core, tile_of, blk_of), 1)
    ch = _ceil_div(counts.max(axis=0), 1)  # per-core max below
    ch = np.ceil(counts.max(axis=0) / P).astype(np.int64)      # [TPC, NBLK]
    ch = np.maximum(ch, (counts.max(axis=0) > 0))              # safety
    ch[:, 0] = np.maximum(ch[:, 0], 1)   # every tile gets >=1 chunk
    TOTCH = int(ch.sum())

    # ---- group/chunk layout (uniform across cores)
    # msg-buffer chunks are laid out block-major (to match the per-block
    # gather calls); dslot columns are laid out TILE-major so each tile's
    # staircase op reads a contiguous dslot slice.  tmbase[t] is the
    # tile-major dslot column base for tile t.
    groups = []
    chunk_cursor = 0   # tile-major dslot column cursor
    idx_cursor16 = 0
    tmbase = {}
    for t in range(TPC):
        tmbase[t] = chunk_cursor
        chunk_cursor += int(ch[t, :].sum())
    assert chunk_cursor == TOTCH
    for g in range(NG):
        tiles = list(range(g * GT, min((g + 1) * GT, TPC)))
        blocks = []
        tile_chunks = {t: [] for t in tiles}   # local msg-chunk ids, in
        local = 0                              # tile-major emission order
        for b in range(NBLK):
            nb = int(sum(ch[t, b] for t in tiles))
            blocks.append((idx_cursor16, nb * P, local))
            for t in tiles:
                for _ in range(int(ch[t, b])):
                    tile_chunks[t].append(local)
                    local += 1
            idx_cursor16 += nb * P // 16
        groups.append({
            "tiles": [(t, tile_chunks[t], tmbase[t]) for t in tiles],
            "blocks": blocks,
            "n_chunks": local,
        })
    TOT16 = idx_cursor16
    CHMAX = max(g["n_chunks"] for g in groups)

    # ---- per-core tensors
    in_maps = []
    # order edges once per core by (tile, block); stable order inside
    for c in range(NCORES):
        m = dst_core == c
        e_row = src_row[m]
        e_tile = tile_of[m]
        e_blk = blk_of[m]
        e_slot = (dst_slot[m] % P).astype(np.float32)
        order = np.lexsort((e_blk, e_tile))
        e_row, e_tile, e_blk, e_slot = (
            e_row[order], e_tile[order], e_blk[order], e_slot[order])
        # bucket boundaries
        key = e_tile * NBLK + e_blk
        bnd = np.searchsorted(key, np.arange(TPC * NBLK + 1))
        idx_vals = np.zeros(TOT16 * 16, dtype=np.int16)
        dslot_vals = np.full(TOTCH * P, -1.0, dtype=np.float32)
        pos = 0   # position in idx_vals stream (gather order: block-major)
        dpos = {t: 0 for t in range(TPC)}   # per-tile dslot chunks written
        for gi, g in enumerate(groups):
            for b_i, (off16, n_idx, local0) in enumerate(g["blocks"]):
                assert pos == off16 * 16
                for (t, _tch, tmb) in g["tiles"]:
                    k = t * NBLK + b_i
                    lo, hi = bnd[k], bnd[k + 1]
                    n_pad = int(ch[t, b_i]) * P
                    seg_idx = np.zeros(n_pad, dtype=np.int16)
                    seg_idx[: hi - lo] = (e_row[lo:hi] % BLKR).astype(np.int16)
                    seg_slot = np.full(n_pad, -1.0, dtype=np.float32)
                    seg_slot[: hi - lo] = e_slot[lo:hi]
                    idx_vals[pos: pos + n_pad] = seg_idx
                    # dslot goes to the tile-major column range
                    d0 = (tmb + dpos[t]) * P
                    dslot_vals[d0: d0 + n_pad] = seg_slot
                    dpos[t] += int(ch[t, b_i])
                    pos += n_pad
        assert pos == TOT16 * 16
        # wrap indices: idx i -> [i % 16, i // 16], replicated to 128 rows
        idx_w = idx_vals.reshape(-1, 16).T.copy()          # [16, TOT16]
        idx_w = np.tile(idx_w, (8, 1))                     # [128, TOT16]
        # dslot: [128, TOTCH] : chunk j partition p = edge j*128+p
        dslot_w = dslot_vals.reshape(TOTCH, P).T.copy()

        nodes = np.arange(c * SR, (c + 1) * SR)
        deg_flat = np.ones(S, dtype=np.float32)
        deg_flat[:SR] = deg[nodes]
        deg_w = deg_flat.reshape(TPC, P).T.copy()
        pool_flat = np.full(S, -1.0, dtype=np.float32)
        pool_flat[:SR] = batch[nodes].astype(np.float32)
        pool_w = pool_flat.reshape(TPC, P).T.copy()

        xT = np.zeros((DIN, S), dtype=np.float32)
        xT[:, :SR] = np.asarray(x[nodes], dtype=np.float32).T

        cnt = np.bincount(batch, minlength=n_graphs).astype(np.float32)
        cnt_flat = np.ones(GP, dtype=np.float32)
        cnt_flat[:n_graphs] = cnt
        cnt_w = cnt_flat.reshape(NGT, P).T.copy()

        iota = np.broadcast_to(
            np.arange(GP, dtype=np.float32)[None, :], (P, GP)).copy()

        in_maps.append({
            "xT": xT, "idx": idx_w, "dslot": dslot_w, "deg": deg_w,
            "pslot": pool_w, "cnt": cnt_w, "iota": iota,
        })

    meta = dict(N=N, DIN=DIN, SR=SR, S=S, TPC=TPC, NBLK=NBLK, BLKR=BLKR,
                GT=GT, NG=NG, GP=GP, NGT=NGT, TOTCH=TOTCH, TOT16=TOT16,
                CHMAX=CHMAX, groups=groups, n_graphs=n_graphs)
    return in_maps, meta


def _build(meta, weights, msg_bufs=3, stage=4, use_bf16=False):
    """Build the SPMD Bass program. weights: dict of numpy arrays (same on
    every core) -> shipped as inputs."""
    SR, S, TPC = meta["SR"], meta["S"], meta["TPC"]
    NBLK, BLKR = meta["NBLK"], meta["BLKR"]
    GP, NGT = meta["GP"], meta["NGT"]
    TOTCH, TOT16, CHMAX = meta["TOTCH"], meta["TOT16"], meta["CHMAX"]
    DIN = meta["DIN"]
    H = weights["W1"].shape[1]
    OUT = weights["Wl"].shape[1]
    n_graphs = meta["n_graphs"]
    has_b = [bool(np.any(weights[k])) for k in ("b1", "b2", "b3")]
    has_bl = bool(np.any(weights["bl"]))

    DT = mybir.dt.bfloat16 if use_bf16 else F32
    nc = bacc.Bacc("TRN2", target_bir_lowering=False, debug=False,
                   num_devices=NCORES, num_swdge_queues=4)

    # ---- I/O tensors
    t_xT = nc.dram_tensor("xT", [DIN, S], F32, kind="ExternalInput")
    t_idx = nc.dram_tensor("idx", [P, TOT16], I16, kind="ExternalInput")
    t_dslot = nc.dram_tensor("dslot", [P, TOTCH], F32, kind="ExternalInput")
    t_deg = nc.dram_tensor("deg", [P, TPC], F32, kind="ExternalInput")
    t_pslot = nc.dram_tensor("pslot", [P, TPC], F32, kind="ExternalInput")
    t_cnt = nc.dram_tensor("cnt", [P, NGT], F32, kind="ExternalInput")
    t_iota = nc.dram_tensor("iota", [P, GP], F32, kind="ExternalInput")
    t_W = {}
    for wn, shp in (("W1", [DIN, H]), ("W2", [H, H]), ("W3", [H, H]),
                    ("Wl", [H, OUT])):
        t_W[wn] = nc.dram_tensor(wn, shp, F32, kind="ExternalInput")
    t_b = {}
    for bn in ("b1", "b2", "b3"):
        t_b[bn] = nc.dram_tensor(bn, [P, H], F32, kind="ExternalInput")
    t_bl = nc.dram_tensor("bl", [P, OUT], F32, kind="ExternalInput")
    t_out = nc.dram_tensor("out", [n_graphs, OUT], F32, kind="ExternalOutput")

    AOP = mybir.AluOpType
    ACT = mybir.ActivationFunctionType

    with tile.TileContext(nc, num_cores=NCORES) as tc:
        with tc.tile_pool(name="const", bufs=1) as cp, \
             tc.tile_pool(name="dram", bufs=1, space="DRAM") as dp:
            # ---- constants to SBUF
            iota_sb = cp.tile([P, GP], F32)
            nc.sync.dma_start(iota_sb[:], t_iota[:])
            idx_sb = cp.tile([P, TOT16], I16)
            nc.sync.dma_start(idx_sb[:], t_idx[:])
            dslot_sb = cp.tile([P, TOTCH], F32)
            nc.sync.dma_start(dslot_sb[:], t_dslot[:])
            deg_sb = cp.tile([P, TPC], F32)
            nc.sync.dma_start(deg_sb[:], t_deg[:])
            pslot_sb = cp.tile([P, TPC], F32)
            nc.sync.dma_start(pslot_sb[:], t_pslot[:])
            cnt_sb = cp.tile([P, NGT], F32)
            nc.sync.dma_start(cnt_sb[:], t_cnt[:])
            W_sb = {}
            for wn, t_w in t_W.items():
                W_sb[wn] = cp.tile(list(t_w.shape), F32, name=f"W_{wn}_sb")
                nc.sync.dma_start(W_sb[wn][:], t_w[:])
            b_sb = {}
            for i, bn in enumerate(("b1", "b2", "b3")):
                if has_b[i]:
                    b_sb[bn] = cp.tile([P, H], F32, name=f"b_{bn}_sb")
                    nc.sync.dma_start(b_sb[bn][:], t_b[bn][:])
            if has_bl:
                bl_sb = cp.tile([P, OUT], F32)
                nc.sync.dma_start(bl_sb[:], t_bl[:])
            ident_sb = cp.tile([P, P], F32)
            from concourse.masks import make_identity
            make_identity(nc, ident_sb[:])

            # dinv = 1/sqrt(deg)  (deg >= 1 always: self loops)
            dinv_sb = cp.tile([P, TPC], F32)
            nc.scalar.sqrt(dinv_sb[:], deg_sb[:])
            nc.vector.reciprocal(dinv_sb[:], dinv_sb[:])
            # cntinv = 1/max(cnt, 1)
            cntinv_sb = cp.tile([P, NGT], F32)
            nc.vector.tensor_scalar_max(cntinv_sb[:], cnt_sb[:], 1.0)
            nc.vector.reciprocal(cntinv_sb[:], cntinv_sb[:])

            # ---- DRAM scratch
            u_shard = dp.tile([S, H], DT)
            u_table = dp.tile([S * NCORES, H], DT)
            pool_dram = dp.tile([GP, H], F32)
            pool_ar = dp.tile([GP, H], F32)

            # ---- layer-1 u: u1 = dinv * (x @ W1), from xT shipped by host
            with tc.tile_pool(name="xTp", bufs=1) as xp, \
                 tc.tile_pool(name="u1w", bufs=4) as u1p, \
                 tc.tile_pool(name="u1ps", bufs=4, space="PSUM") as u1ps:
                xT_sb = xp.tile([DIN, S], F32)
                nc.sync.dma_start(xT_sb[:], t_xT[:])
                for t in range(TPC):
                    ps = u1ps.tile([P, H], F32, tag="ps")
                    nc.tensor.matmul(ps[:], lhsT=xT_sb[:, t * P:(t + 1) * P],
                                     rhs=W_sb["W1"][:], start=True, stop=True)
                    u_sb = u1p.tile([P, H], DT, tag="u")
                    nc.scalar.activation(u_sb[:], ps[:], ACT.Copy,
                                         scale=dinv_sb[:, t:t + 1])
                    nc.sync.dma_start(u_shard[t * P:(t + 1) * P, :], u_sb[:])

            # ---- main layer loop
            with tc.tile_pool(name="msg", bufs=msg_bufs) as mp, \
                 tc.tile_pool(name="stair", bufs=2) as sp, \
                 tc.tile_pool(name="work", bufs=3) as wp, \
                 tc.tile_pool(name="hps", bufs=2, space="PSUM") as hps, \
                 tc.tile_pool(name="tps", bufs=2, space="PSUM") as tps, \
                 tc.tile_pool(name="ups", bufs=2, space="PSUM") as ups, \
                 tc.tile_pool(name="pps", bufs=1, space="PSUM") as pps:
                pool_ps = pps.tile([P, GP], F32)
                n_layers = 3 if stage >= 3 else (1 if stage >= 1.5 else 0)
                for li in range(n_layers):
                    # AllGather this layer's u
                    nc.gpsimd.collective_compute(
                        "AllGather", AOP.bypass,
                        replica_groups=[list(range(NCORES))],
                        ins=[u_shard.opt()], outs=[u_table.opt()],
                    )
                    W_next = ("W2", "W3", None)[li]
                    for g in meta["groups"]:
                        nch = g["n_chunks"]
                        msg = mp.tile([P, CHMAX, H], DT, tag="msg")
                        for b_i, (off16, n_idx, local0) in enumerate(g["blocks"]):
                            if n_idx == 0 or stage < 2:
                                continue
                            nbch = n_idx // P
                            blk_hi = min((b_i + 1) * BLKR, S * NCORES)
                            nc.gpsimd.dma_gather(
                                out_ap=msg[:, local0:local0 + nbch, :],
                                in_ap=u_table[b_i * BLKR:blk_hi, :],
                                idxs_ap=idx_sb[:, off16:off16 + n_idx // 16],
                                num_idxs=n_idx,
                                num_idxs_reg=n_idx,
                                elem_size=H,
                                single_packet=False,
                                queue_num=b_i % 4,
                            )
                        for (t, tch, tmb) in g["tiles"]:
                            if not tch or stage < 3:
                                continue
                            ntc = len(tch)
                            # one-hot staircases for all chunks of this tile
                            stair = sp.tile([P, ntc, P], DT, tag="st")
                            nc.vector.tensor_tensor(
                                out=stair[:],
                                in0=iota_sb[:, 0:P].unsqueeze(1)
                                    .to_broadcast([P, ntc, P]),
                                in1=dslot_sb[:, tmb:tmb + ntc].unsqueeze(2)
                                    .to_broadcast([P, ntc, P]),
                                op=AOP.is_equal,
                            )
                            ps_h = hps.tile([P, H], F32, tag="h")
                            for k, lc in enumerate(tch):
                                nc.tensor.matmul(
                                    ps_h[:], lhsT=stair[:, k, :],
                                    rhs=msg[:, lc, :],
                                    start=(k == 0), stop=(k == ntc - 1))
                            # self-loop: add this tile's own u (local read)
                            u_loc = wp.tile([P, H], DT, tag="uloc")
                            nc.sync.dma_start(u_loc[:],
                                              u_shard[t * P:(t + 1) * P, :])
                            tmp = wp.tile([P, H], F32, tag="tmp")
                            nc.vector.tensor_tensor(
                                out=tmp[:], in0=ps_h[:], in1=u_loc[:],
                                op=AOP.add)
                            h_sb = wp.tile([P, H], F32, tag="h")
                            if has_b[li]:
                                nc.vector.tensor_scalar_mul(
                                    tmp[:], tmp[:], dinv_sb[:, t:t + 1])
                                nc.vector.tensor_tensor(
                                    out=tmp[:], in0=tmp[:],
                                    in1=b_sb[("b1", "b2", "b3")[li]][:],
                                    op=AOP.add)
                                nc.scalar.activation(h_sb[:], tmp[:], ACT.Relu)
                            else:
                                nc.scalar.activation(
                                    h_sb[:], tmp[:], ACT.Relu,
                                    scale=dinv_sb[:, t:t + 1])
                            if W_next is not None:
                                ps_t = tps.tile([P, P], F32, tag="t")
                                nc.tensor.transpose(ps_t[:], h_sb[:],
                                                    ident_sb[:])
                                hT_sb = wp.tile([P, P], F32, tag="ht")
                                nc.vector.tensor_copy(hT_sb[:], ps_t[:])
                                ps_u = ups.tile([P, H], F32, tag="u")
                                nc.tensor.matmul(ps_u[:], lhsT=hT_sb[:],
                                                 rhs=W_sb[W_next][:],
                                                 start=True, stop=True)
                                u_sb = wp.tile([P, H], DT, tag="u")
                                nc.scalar.activation(
                                    u_sb[:], ps_u[:], ACT.Copy,
                                    scale=dinv_sb[:, t:t + 1])
                                nc.sync.dma_start(
                                    u_shard[t * P:(t + 1) * P, :], u_sb[:])
                            else:
                                # pool: one-hot over global graph slots
                                stp = sp.tile([P, GP], F32, tag="stp")
                                nc.vector.tensor_tensor(
                                    out=stp[:], in0=iota_sb[:],
                                    in1=pslot_sb[:, t:t + 1]
                                        .to_broadcast([P, GP]),
                                    op=AOP.is_equal)
                                # start=True clears has_written for the WHOLE
                                # 2KB psum bank -> only the first matmul per
                                # bank may set it (4 halves of 128 f32 / bank).
                                for hh in range(NGT):
                                    nc.tensor.matmul(
                                        pool_ps[:, hh * P:(hh + 1) * P],
                                        lhsT=stp[:, hh * P:(hh + 1) * P],
                                        rhs=h_sb[:],
                                        start=(t == 0 and hh % 4 == 0),
                                        stop=(t == TPC - 1),
                                        skip_group_check=True)

                if stage < 4:
                    z_sb = wp.tile([P, OUT], F32, tag="o")
                    nc.vector.memset(z_sb[:], 0.0)
                    nc.sync.dma_start(t_out[0:min(P, n_graphs), :],
                                      z_sb[:min(P, n_graphs), :])
                else:
                    # ---- pool wrap-up: PSUM -> SBUF -> DRAM -> AllReduce
                    poolacc = wp.tile([P, GP], F32, tag="pa")
                    nc.vector.tensor_copy(poolacc[:], pool_ps[:])
                    nc.sync.dma_start(
                        pool_dram[:].rearrange("(h p) f -> p h f", p=P),
                        poolacc[:].rearrange("p (h f) -> p h f", h=NGT))
                    nc.gpsimd.collective_compute(
                        "AllReduce", AOP.add,
                        replica_groups=[list(range(NCORES))],
                        ins=[pool_dram.opt()], outs=[pool_ar.opt()],
                    )
                    # ---- final linear on pooled means (replicated)
                    for gt in range(NGT):
                        pt = wp.tile([P, H], F32, tag="pt")
                        nc.sync.dma_start(pt[:],
                                          pool_ar[gt * P:(gt + 1) * P, :])
                        nc.vector.tensor_scalar_mul(pt[:], pt[:],
                                                    cntinv_sb[:, gt:gt + 1])
                        ps_t = tps.tile([P, P], F32, tag="t")
                        nc.tensor.transpose(ps_t[:], pt[:], ident_sb[:])
                        ptT = wp.tile([P, P], F32, tag="ptT")
                        nc.vector.tensor_copy(ptT[:], ps_t[:])
                        ps_o = ups.tile([P, OUT], F32, tag="u")
                        nc.tensor.matmul(ps_o[:], lhsT=ptT[:],
                                         rhs=W_sb["Wl"][:],
                                         start=True, stop=True)
                        o_sb = wp.tile([P, OUT], F32, tag="o")
                        if has_bl:
                            nc.vector.tensor_tensor(out=o_sb[:], in0=ps_o[:],
                                                    in1=bl_sb[:], op=AOP.add)
                        else:
                            nc.vector.tensor_copy(o_sb[:], ps_o[:])
                        rows = min(P, n_graphs - gt * P)
                        nc.sync.dma_start(t_out[gt * P:gt * P + rows, :],
                                          o_sb[:rows, :])

    nc.finalize()
    return nc


def kernel(x, edge_index, batch, W1, b1, W2, b2, W3, b3, Wl, bl,
           group_tiles=4, trace=False, n_graphs=1000, stage=4, use_bf16=True):
    weights = dict(W1=np.asarray(W1, np.float32), b1=np.asarray(b1, np.float32),
                   W2=np.asarray(W2, np.float32), b2=np.asarray(b2, np.float32),
                   W3=np.asarray(W3, np.float32), b3=np.asarray(b3, np.float32),
                   Wl=np.asarray(Wl, np.float32), bl=np.asarray(bl, np.float32))
    in_maps, meta = _preprocess(np.asarray(x, np.float32),
                                np.asarray(edge_index), np.asarray(batch),
                                n_graphs, group_tiles=group_tiles)
    nc = _build(meta, weights, stage=stage, use_bf16=use_bf16)
    # broadcast weight tensors (same on every core)
    H = weights["W1"].shape[1]
    OUT = weights["Wl"].shape[1]
    for m in in_maps:
        for wn in ("W1", "W2", "W3", "Wl"):
            m[wn] = weights[wn]
        for bn in ("b1", "b2", "b3"):
            m[bn] = np.broadcast_to(weights[bn][None, :], (P, H)).copy()
        m["bl"] = np.broadcast_to(weights["bl"][None, :], (P, OUT)).copy()
    res = run_bass_kernel_spmd(nc, in_maps, core_ids=list(range(NCORES)),
                               trace=trace)
    kernel.last_result = res
    return res.results[0]["out"][:n_graphs].astype(np.float32)

